# revision 2
# baseline (speedup 1.0000x reference)
"""HGT kernel v1 for 8 trn2 NeuronCores — bf16 + host-sel + batched DMA.

Changes vs baseline:
  - bf16 data everywhere (xs, kv/q tables, collectives, scatters); fp32 only
    for PSUM, logits, reciprocal, and final y.
  - Host-built selection matrices (bf16) replace on-device transpose+is_equal.
  - Batched indirect DMA (BB tiles per gather/scatter) and batched DVE ops
    (VB tiles per instruction) to amortize fixed overheads.
  - Dense phases use DMA-transpose loads for lhsT (no PE transposes), bias via
    K=1 ones-row matmul, PSUM evacuation on ACT/DVE alternating.
  - Node counts padded to multiples of 128; phase D before phase C so the
    ReduceScatter overlaps phase-C edge tiles.
"""

import math
import os
import numpy as np
import ml_dtypes

try:
    import concourse  # noqa
except ImportError:
    import sys
    sys.path.insert(0, "/opt/trn_rl_repo")

from concourse import bacc, bass, mybir, tile
from concourse.bass import IndirectOffsetOnAxis
from concourse.bass_utils import run_bass_kernel_spmd

P = 128
H, DH, HID, IN_DIM, OUT_DIM = 8, 32, 256, 768, 128
L = 2
NU_F, NM_F, NR_F = 50000, 20000, 200000
C = 8
NU, NM, NR = NU_F // C, NM_F // C, NR_F // C          # 6250, 2500, 25000
NU_P, NM_P, NR_P = 6272, 2560, 25088                  # padded to %128
AG_BLK = NM_P + NU_P                                  # 8832 (movie first)
UBLK = 6400
BB = int(os.environ.get("KBB", "8"))   # tiles per indirect-DMA batch
VB = 4                                 # tiles per DVE instruction
F32 = mybir.dt.float32
BF = mybir.dt.bfloat16
I32 = mybir.dt.int32
AF = mybir.ActivationFunctionType
ALU = mybir.AluOpType
BF_NP = ml_dtypes.bfloat16

LAST_RESULTS = None


# ---------------------------------------------------------------- host prep

def _fold_weights(inp):
    """Fold a_rel/m_rel/p_rel into Wk/Wv per src type; sigmoid(skip) into Wa.
    All outputs cast to bf16 (biases as [1, w] rows for the ones-matmul)."""
    Wk, bk = inp["Wk"], inp["bk"]
    Wq, bq = inp["Wq"], inp["bq"]
    Wv, bv = inp["Wv"], inp["bv"]
    Wa, ba = inp["Wa"], inp["ba"]
    a_rel, m_rel, p_rel, skip = inp["a_rel"], inp["m_rel"], inp["p_rel"], inp["skip"]
    s_of_e = {0: 1, 1: 0, 2: 2}  # edge type -> src node type
    out = {}
    def b16(a):
        return np.ascontiguousarray(a).astype(BF_NP)
    for l in range(L):
        for e in range(3):
            s = s_of_e[e]
            wk_eff = np.empty((HID, HID), np.float32)
            bk_eff = np.empty((HID,), np.float32)
            wv_eff = np.empty((HID, HID), np.float32)
            bv_eff = np.empty((HID,), np.float32)
            for h in range(H):
                sl = slice(h * DH, (h + 1) * DH)
                sc = float(p_rel[l, e, h]) / math.sqrt(DH)
                wk_eff[:, sl] = (Wk[l, s][:, sl] @ a_rel[l, e, h]) * sc
                bk_eff[sl] = (bk[l, s][sl] @ a_rel[l, e, h]) * sc
                wv_eff[:, sl] = Wv[l, s][:, sl] @ m_rel[l, e, h]
                bv_eff[sl] = bv[l, s][sl] @ m_rel[l, e, h]
            out[f"wkv_t{s}_l{l}"] = b16(np.concatenate([wk_eff, wv_eff], 1))
            out[f"bkv_t{s}_l{l}"] = b16(np.concatenate([bk_eff, bv_eff]).reshape(1, 512))
        for t in (0, 2):
            out[f"wq_t{t}_l{l}"] = b16(Wq[l, t])
            out[f"bq_t{t}_l{l}"] = b16(np.asarray(bq[l, t]).reshape(1, HID))
        for t in range(3):
            g = 1.0 / (1.0 + math.exp(-float(skip[l, t])))
            out[f"omg_l{l}_t{t}"] = 1.0 - g
            if t != 1:
                out[f"wa_t{t}_l{l}"] = b16(np.asarray(Wa[l, t]) * g)
            out[f"ba_t{t}_l{l}"] = b16((np.asarray(ba[l, t]) * g).reshape(1, HID))
    out["w1"] = b16(inp["W1"])
    out["b1"] = b16(np.asarray(inp["b1"]).reshape(1, HID))
    out["w2"] = b16(inp["W2"])
    out["b2"] = b16(np.asarray(inp["b2"]).reshape(1, OUT_DIM))
    return out


def _pack(group_ids, payload_cols, pad_vals, dtypes):
    """Pack edges (sorted by group) into 128-slot tiles; groups never straddle
    a tile. Returns (T, [T,P] arrays)."""
    n = len(group_ids)
    if n == 0:
        return 0, [np.full((0, P), pv, dt) for pv, dt in zip(pad_vals, dtypes)]
    order = np.argsort(group_ids, kind="stable")
    g = group_ids[order]
    uniq, counts = np.unique(g, return_counts=True)
    ng = len(uniq)
    tile_id = np.empty(ng, np.int64)
    slot0 = np.empty(ng, np.int64)
    cur_t, fill = 0, 0
    cl = counts.tolist()
    for i in range(ng):
        c = cl[i]
        assert c <= P, f"group degree {c} > {P}"
        if fill + c > P:
            cur_t += 1
            fill = 0
        tile_id[i] = cur_t
        slot0[i] = fill
        fill += c
    T = cur_t + 1
    gi = np.repeat(np.arange(ng), counts)
    starts = np.cumsum(counts) - counts
    within = np.arange(n) - starts[gi]
    tid = tile_id[gi]
    slot = slot0[gi] + within
    outs = []
    for col, pv, dt in zip(payload_cols, pad_vals, dtypes):
        arr = np.full((T, P), pv, dtype=dt)
        arr[tid, slot] = col[order].astype(dt)
        outs.append(arr)
    return T, outs


def _sel_from_keys(keys):
    """keys: [T, P] int64 (pad slots must hold unique negatives).
    Returns [P, T*P] bf16 with tile t in cols [t*P,(t+1)*P)."""
    T = keys.shape[0]
    sel = (keys[:, :, None] == keys[:, None, :])
    return np.ascontiguousarray(
        sel.transpose(1, 0, 2).reshape(P, T * P).astype(BF_NP))


def _prep_edges(inp):
    """Per-core packed edge tiles + host-built selection matrices."""
    src_mr, dst_mr = inp["src_mr"], inp["dst_mr"]
    src_ur, dst_ur = inp["src_ur"], inp["dst_ur"]
    src_ru, dst_ru = inp["src_ru"], inp["dst_ru"]
    pad_slots = -1 - np.arange(P, dtype=np.int64)  # unique negative per slot

    # phase C: review-dst edges (mr type0 + ur type1), sharded by dst shard
    sm = (src_mr.astype(np.int64) // NM) * AG_BLK + (src_mr % NM)
    su = (src_ur.astype(np.int64) // NU) * AG_BLK + NM_P + (src_ur % NU)
    src_all = np.concatenate([sm, su])
    dst_all = np.concatenate([dst_mr, dst_ur]).astype(np.int64)
    typ_all = np.concatenate(
        [np.zeros(len(sm), np.int64), np.ones(len(su), np.int64)])
    csp = []
    pvC = [0, 0, NR_P, -1, -1]
    dtC = [np.int32, np.int32, np.int32, np.int64, np.int64]
    for c in range(C):
        m = (dst_all // NR) == c
        dl = dst_all[m] % NR
        cols = [src_all[m], dl, dl, dl * 2 + typ_all[m], dl]
        csp.append(_pack(dl, cols, pvC, dtC))
    T_C = max(max(t for t, _ in csp), 1)
    T_C = ((T_C + BB - 1) // BB) * BB
    cs = []
    for _, arrs in csp:
        padded = []
        for a, pv, dt in zip(arrs, pvC, dtC):
            full = np.full((T_C, P), pv, dtype=dt)
            full[: a.shape[0]] = a
            padded.append(full)
        # pad slots get unique negative keys (keys are cols 3,4)
        for ki in (3, 4):
            k = padded[ki]
            pad = k < 0
            k[pad] = np.broadcast_to(pad_slots, (T_C, P))[pad]
        cs.append({
            "src": np.ascontiguousarray(padded[0].T),
            "qi": np.ascontiguousarray(padded[1].T),
            "dst": np.ascontiguousarray(padded[2].T),
            "sden": _sel_from_keys(padded[3]),
            "sdst": _sel_from_keys(padded[4]),
        })

    # phase D: ru edges (review->user), sharded by src shard
    s64, d64 = src_ru.astype(np.int64), dst_ru.astype(np.int64)
    flat = (d64 // NU) * UBLK + (d64 % NU)
    qg = (d64 // NU) * NU_P + (d64 % NU)
    rup = []
    pvD = [0, 0, UBLK * C, -1]
    dtD = [np.int32, np.int32, np.int32, np.int64]
    for c in range(C):
        m = (s64 // NR) == c
        cols = [s64[m] % NR, qg[m], flat[m], flat[m]]
        rup.append(_pack(flat[m], cols, pvD, dtD))
    T_D = max(max(t for t, _ in rup), 1)
    T_D = ((T_D + BB - 1) // BB) * BB
    ru = []
    for _, arrs in rup:
        padded = []
        for a, pv, dt in zip(arrs, pvD, dtD):
            full = np.full((T_D, P), pv, dtype=dt)
            full[: a.shape[0]] = a
            padded.append(full)
        k = padded[3]
        pad = k < 0
        k[pad] = np.broadcast_to(pad_slots, (T_D, P))[pad]
        ru.append({
            "src": np.ascontiguousarray(padded[0].T),
            "qi": np.ascontiguousarray(padded[1].T),
            "dst": np.ascontiguousarray(padded[2].T),
            "sel": _sel_from_keys(padded[3]),
        })
    return T_C, cs, T_D, ru
